# revision 20
# baseline (speedup 1.0000x reference)
"""Trainium2 Bass kernel for nn_KernelGraphCalcLayer (GNN message passing).

Computation (per batch b):
    h = relu(node_feats @ weight + bias)            # (N, OUT_DIM)
    h = h.reshape(N, K, DK)
    out[n, k, d] = sum_m adj[k, n, m] * h[m, k, d]  # per-kernel dense aggregation

Sharding: batch dim (64) split across 8 NeuronCores, 8 batches per core.
No cross-device communication.

v5 dataflow (HBM floor: 22.4MB reads + 2.1MB bf16 writes @~358GB/s):
  - adj is the only SWDGE traffic: cast fp32->bf16 in flight, (p c)
    row-pair packing -> 2KB descriptors, two 4-kernel chunks per batch,
    ~6 batches of prefetch. GpSimd carries nothing but adj emission
    (each SWDGE dma_start costs ~1.15us of Q7 descriptor generation)
    plus the identity build slotted between the first two chunks.
  - x/W/bias load fp32 on the Sync HWDGE queue, W first (its casts gate
    the first linear). x casts fp32->bf16 run on ACT, W/bias on DVE.
  - PE warmup: ~34 back-to-back dummy matmuls guarantee one fully-busy
    HAM window so the clock gate lifts (1.2->2.4GHz) before real work.
  - All PE work in bf16 (FWL-eligible): 8 xT + 32 adjT transposes,
    2 bias-seed + 8 linear matmuls, 32 aggregation matmuls per batch.
  - Drains: DVE takes the bf16 transpose drains, ACT takes relu, out
    copies (cast to bf16), and x casts.
  - Output is stored as bf16 (host converts back to fp32): halves store
    traffic; quantization (~0.4%) is far inside the 2e-2 gate.
  - agg(b-1) is slotted between adjT-A(b) and linear(b) so the PE never
    waits on its own batch's relu/drains and the last-batch tail is
    only adjT-B + agg + drain + store.
"""

import numpy as np

import concourse.bass as bass
import concourse.mybir as mybir
from concourse import bacc
import concourse.tile as tile
from concourse.bass_utils import run_bass_kernel_spmd
from concourse.masks import make_identity

B, N, IN_DIM, OUT_DIM, K = 64, 256, 512, 512, 8
DK = OUT_DIM // K
N_CORES = 8
BPC = B // N_CORES  # batches per core

FP32 = mybir.dt.float32
CDT = mybir.dt.bfloat16
P = 128

WARMUP_MM = 34  # guarantees one fully-busy HAM window at cold clock

_compiled = {}


def _build(cdt=CDT):
    nc = bacc.Bacc("TRN2", target_bir_lowering=False, debug=False)
    x_ap = nc.dram_tensor("node_feats", [BPC, N, IN_DIM], FP32, kind="ExternalInput").ap()
    adj_ap = nc.dram_tensor("adj", [BPC, K, N, N], FP32, kind="ExternalInput").ap()
    w_ap = nc.dram_tensor("weight", [IN_DIM, OUT_DIM], FP32, kind="ExternalInput").ap()
    b_ap = nc.dram_tensor("bias", [OUT_DIM], FP32, kind="ExternalInput").ap()
    out_ap = nc.dram_tensor("out", [BPC, N, OUT_DIM], CDT, kind="ExternalOutput").ap()

    NC2 = N // P       # 2 node chunks of 128
    IC4 = IN_DIM // P  # 4 input-feature chunks
    KH = K // 2        # kernels per adj half-load
    KPH = K // 4       # k-pairs per half

    # adj: partition p holds rows {2p, 2p+1} (c in {0,1}) -> one 2KB
    # contiguous descriptor per (partition, k)
    adj_v = adj_ap.rearrange("b k (p c) m -> b p k c m", c=2)
    # x: natural node chunks (c p): partition p of chunk c = node 128c+p
    x_v = x_ap.rearrange("b (c p) i -> b p c i", p=P)
    # out: partition p holds rows {2p, 2p+1} -> 2KB bf16 per partition
    out_v = out_ap.rearrange("b (p c) o -> b p c o", c=2)

    with tile.TileContext(nc) as tc:
        with (
            tc.tile_pool(name="singles", bufs=1) as singles,
            tc.tile_pool(name="p_adj", bufs=12) as p_adj,
            tc.tile_pool(name="p_x", bufs=4) as p_x,
            tc.tile_pool(name="p_xt", bufs=3) as p_xt,
            tc.tile_pool(name="p_h", bufs=6) as p_h,
            tc.tile_pool(name="p_at", bufs=10) as p_at,
            tc.tile_pool(name="p_out", bufs=4) as p_out,
            tc.tile_pool(name="ps_t", bufs=3, space=bass.MemorySpace.PSUM) as ps_t,
            tc.tile_pool(name="ps_h", bufs=2, space=bass.MemorySpace.PSUM) as ps_h,
            tc.tile_pool(name="ps_o", bufs=2, space=bass.MemorySpace.PSUM) as ps_o,
        ):
            # --- Sync HWDGE: W/bias only (their casts gate linear(0));
            # stores ride behind them later ---
            w_f32 = [singles.tile([P, OUT_DIM], FP32, name=f"wf{ic}")
                     for ic in range(IC4)]
            for ic in range(IC4):
                nc.sync.dma_start(out=w_f32[ic][:],
                                  in_=w_ap[ic * P:(ic + 1) * P, :])
            bias_f32 = singles.tile([1, OUT_DIM], FP32)
            nc.sync.dma_start(out=bias_f32[:], in_=b_ap[None, :])

            # --- GpSimd SWDGE: one FIFO carrying x(b), adjA(b), adjB(b)
            # per batch — arrival order exactly matches consumption order,
            # so the PE is never starved and never races ahead. Identity
            # build is slotted after the first adj chunk. ---
            adj_sbs = [[None, None] for _ in range(BPC)]
            x_sbs = []
            id_c = singles.tile([P, P], cdt)

            def load_adj(b, hf):
                at = p_adj.tile([P, KH * 2 * N], cdt, tag="adj",
                                name=f"a{b}_{hf}")
                nc.gpsimd.dma_start(
                    out=at[:], in_=adj_v[b, :, hf * KH:(hf + 1) * KH])
                adj_sbs[b][hf] = at

            def load_x(b):
                xt = p_x.tile([P, NC2 * IN_DIM], cdt, tag="x", name=f"x{b}")
                nc.gpsimd.dma_start(out=xt[:], in_=x_v[b])
                x_sbs.append(xt)

            load_x(0)
            load_adj(0, 0)
            make_identity(nc, id_c[:])
            load_adj(0, 1)
            for b in range(1, BPC):
                load_x(b)
                load_adj(b, 0)
                load_adj(b, 1)

            # --- DVE: warmup tile + ones row + W/bias casts ---
            warm = singles.tile([P, P], cdt)
            nc.vector.memset(warm[:], 0.125)
            ones_row = singles.tile([1, P], cdt)
            nc.vector.memset(ones_row[:], 1.0)
            w_sb = [singles.tile([P, OUT_DIM], cdt, name=f"w{ic}")
                    for ic in range(IC4)]
            for ic in range(IC4):
                nc.vector.tensor_copy(w_sb[ic][:], w_f32[ic][:])
            bias_c = singles.tile([1, OUT_DIM], cdt)
            nc.vector.tensor_copy(bias_c[:], bias_f32[:])

            # --- PE warmup (borrows the ps_o ring) ---
            pw = [ps_o.tile([P, OUT_DIM], FP32, tag="pso", name=f"wm{i}")
                  for i in range(2)]
            for i in range(WARMUP_MM):
                nc.tensor.matmul(pw[i % 2][:, :P], warm[:], warm[:],
                                 start=True, stop=True)

            pend = [None] * BPC

            def emit_xt_linear(b):
                """xT transposes for batch b"""
                x_sb = x_sbs[b]
                pt = ps_t.tile([P, NC2 * IC4 * P], cdt, tag="pst",
                               name=f"ptx{b}")
                for c in range(NC2):
                    for ic in range(IC4):
                        nc.tensor.transpose(
                            pt[:, (c * IC4 + ic) * P:(c * IC4 + ic + 1) * P],
                            x_sb[:, c * IN_DIM + ic * P:
                                 c * IN_DIM + (ic + 1) * P],
                            id_c[:])
                xt_sb = p_xt.tile([P, NC2 * IC4 * P], cdt, tag="xT",
                                  name=f"xT{b}")
                nc.vector.tensor_copy(xt_sb[:], pt[:])
                return xt_sb

            def emit_linear(b, xt_sb):
                h_sb = []
                for c in range(NC2):
                    ph = ps_h.tile([P, OUT_DIM], FP32, tag="psh",
                                   name=f"ph{b}_{c}")
                    nc.tensor.matmul(ph[:], ones_row[:], bias_c[:],
                                     start=True, stop=False)
                    for ic in range(IC4):
                        nc.tensor.matmul(
                            ph[:], xt_sb[:, (c * IC4 + ic) * P:
                                         (c * IC4 + ic + 1) * P],
                            w_sb[ic][:], start=False, stop=(ic == IC4 - 1))
                    ht = p_h.tile([P, OUT_DIM], cdt, tag="h", name=f"h{b}_{c}")
                    nc.scalar.activation(ht[:], ph[:],
                                         mybir.ActivationFunctionType.Relu)
                    h_sb.append(ht)
                return h_sb

            def emit_adjt(b, half):
                """adjT transposes for k-pairs of one half (2 kp per half)"""
                aT = []
                for kph in range(KPH):
                    kp = half * KPH + kph
                    pt = ps_t.tile([P, 8 * P], cdt, tag="pst",
                                   name=f"pta{b}_{kp}")
                    for kk in range(2):
                        k = kp * 2 + kk
                        a_sb = adj_sbs[b][k // KH]
                        klocal = k % KH
                        for c in range(2):
                            for mch in range(NC2):
                                nc.tensor.transpose(
                                    pt[:, (kk * 4 + c * NC2 + mch) * P:
                                       (kk * 4 + c * NC2 + mch + 1) * P],
                                    a_sb[:, klocal * 2 * N + c * N + mch * P:
                                         klocal * 2 * N + c * N + (mch + 1) * P],
                                    id_c[:])
                    t = p_at.tile([P, 8 * P], cdt, tag="aT",
                                  name=f"aT{b}_{kp}")
                    nc.vector.tensor_copy(t[:], pt[:])
                    aT.append(t)
                return aT

            po_pend = [None] * BPC

            def emit_agg_half(b, half):
                """aggregation matmuls for k of one half"""
                h_sb, aT = pend[b]
                if po_pend[b] is None:
                    po_pend[b] = [ps_o.tile([P, OUT_DIM], FP32, tag="pso",
                                            name=f"po{b}_{c}")
                                  for c in range(2)]
                po = po_pend[b]
                for k in range(half * KH, (half + 1) * KH):
                    kp, kk = k // 2, k % 2
                    for c in range(2):
                        for mch in range(NC2):
                            nc.tensor.matmul(
                                po[c][:, k * DK:(k + 1) * DK],
                                aT[kp][:, (kk * 4 + c * NC2 + mch) * P:
                                       (kk * 4 + c * NC2 + mch + 1) * P],
                                h_sb[mch][:, k * DK:(k + 1) * DK],
                                start=(mch == 0), stop=(mch == NC2 - 1))

            def emit_store(b):
                po = po_pend[b]
                ot = p_out.tile([P, 2 * OUT_DIM], cdt, tag="o", name=f"o{b}")
                for c in range(2):
                    nc.scalar.copy(ot[:, c * OUT_DIM:(c + 1) * OUT_DIM],
                                   po[c][:])
                nc.sync.dma_start(out=out_v[b], in_=ot[:])

            def emit_agg(b):
                emit_agg_half(b, 0)
                emit_agg_half(b, 1)
                emit_store(b)

            def emit_front(b, mid=None, tail_split=False):
                """batch b's front, with `mid` (agg of b-1) slotted between
                the first adjT half and the linear. With tail_split, the
                A-half aggregation of b itself runs before the B-half
                transposes, shortening the last-batch tail."""
                xt_sb = emit_xt_linear(b)
                aT = emit_adjt(b, 0)
                if mid is not None:
                    mid()
                h_sb = emit_linear(b, xt_sb)
                pend[b] = (h_sb, aT)
                if tail_split:
                    emit_agg_half(b, 0)
                aT += emit_adjt(b, 1)
                pend[b] = (h_sb, aT)

            emit_front(0)
            for b in range(1, BPC - 1):
                emit_front(b, mid=lambda bb=b - 1: emit_agg(bb))
            emit_front(BPC - 1, mid=lambda: emit_agg(BPC - 2),
                       tail_split=True)
            emit_agg_half(BPC - 1, 1)
            emit_store(BPC - 1)

    nc.compile()
    return nc


def _get_nc():
    if "nc" not in _compiled:
        _compiled["nc"] = _build()
    return _compiled["nc"]


def _run(inputs, trace=False, trace_cores=None):
    nc = _get_nc()
    node_feats = np.ascontiguousarray(inputs["node_feats"], dtype=np.float32)
    adj = np.ascontiguousarray(inputs["adj"], dtype=np.float32)
    weight = np.ascontiguousarray(inputs["weight"], dtype=np.float32)
    bias = np.ascontiguousarray(inputs["bias"], dtype=np.float32)
    in_maps = []
    for c in range(N_CORES):
        sl = slice(c * BPC, (c + 1) * BPC)
        in_maps.append({
            "node_feats": node_feats[sl],
            "adj": adj[sl],
            "weight": weight,
            "bias": bias,
        })
    res = run_bass_kernel_spmd(
        nc, in_maps, core_ids=list(range(N_CORES)),
        trace=trace, trace_cores=trace_cores)
    out = np.concatenate(
        [np.asarray(res.results[c]["out"]).astype(np.float32)
         for c in range(N_CORES)], axis=0)
    return out.reshape(B, N, OUT_DIM), res


def kernel(**inputs) -> np.ndarray:
    return _run(inputs, trace=False)[0]


# revision 21
# speedup vs baseline: 1.0549x; 1.0549x over previous
"""Trainium2 Bass kernel for nn_KernelGraphCalcLayer (GNN message passing).

Computation (per batch b):
    h = relu(node_feats @ weight + bias)            # (N, OUT_DIM)
    h = h.reshape(N, K, DK)
    out[n, k, d] = sum_m adj[k, n, m] * h[m, k, d]  # per-kernel dense aggregation

Sharding: batch dim (64) split across 8 NeuronCores, 8 batches per core.
No cross-device communication.

v7 dataflow (HBM floor: 22.4MB reads + 2.1MB bf16 writes @~358GB/s):
  - ONE SWDGE FIFO carries every load, in consumption order:
    W, bias, x0, adjA0, adjB0, x1, adjA1, ... — all cast fp32->bf16 in
    flight. adj uses (p c) row-pair packing (2KB descriptors); W loads
    as a single [128, (ic o)] transfer. Sync HWDGE carries only the
    8 output stores. ~6 batches of adj prefetch.
  - PE warmup: ~34 back-to-back dummy matmuls lift the HAM clock gate
    (1.2->2.4GHz) before real work; a short second burst bridges the
    slot-0 gap until the first adj chunk lands.
  - Per-batch PE slot (all bf16, FWL-eligible): the arrival-gated adjT
    transposes sit at the END of the slot, with the previous batch's
    aggregation as the bridge work:
      [xT(b), aggA(b-1), linear(b), aggB(b-1)+store(b-1),
       adjTA(b), adjTB(b)]
    so the PE never stalls on a fresh relu/drain, and the last batch's
    tail is only adjTB + agg + drain + store.
  - Drains: DVE takes transpose drains + po[1], ACT takes relu + po[0]
    (parallel output drain). Output is stored bf16 (host converts to
    fp32): halves store traffic; quantization ~0.4% vs the 2e-2 gate.
"""

import numpy as np

import concourse.bass as bass
import concourse.mybir as mybir
from concourse import bacc
import concourse.tile as tile
from concourse.bass_utils import run_bass_kernel_spmd
from concourse.masks import make_identity

B, N, IN_DIM, OUT_DIM, K = 64, 256, 512, 512, 8
DK = OUT_DIM // K
N_CORES = 8
BPC = B // N_CORES  # batches per core

FP32 = mybir.dt.float32
CDT = mybir.dt.bfloat16
P = 128

WARMUP_MM = 34  # guarantees one fully-busy HAM window at cold clock
BRIDGE_MM = 8   # 512-free dummies bridging slot-0 until adjA0 lands

_compiled = {}


def _build(cdt=CDT):
    nc = bacc.Bacc("TRN2", target_bir_lowering=False, debug=False)
    x_ap = nc.dram_tensor("node_feats", [BPC, N, IN_DIM], FP32, kind="ExternalInput").ap()
    adj_ap = nc.dram_tensor("adj", [BPC, K, N, N], FP32, kind="ExternalInput").ap()
    w_ap = nc.dram_tensor("weight", [IN_DIM, OUT_DIM], FP32, kind="ExternalInput").ap()
    b_ap = nc.dram_tensor("bias", [OUT_DIM], FP32, kind="ExternalInput").ap()
    out_ap = nc.dram_tensor("out", [BPC, N, OUT_DIM], CDT, kind="ExternalOutput").ap()

    NC2 = N // P       # 2 node chunks of 128
    IC4 = IN_DIM // P  # 4 input-feature chunks
    KH = K // 2        # kernels per adj half-load
    KPH = K // 4       # k-pairs per half

    # adj: partition p holds rows {2p, 2p+1} (c in {0,1}) -> one 2KB
    # contiguous descriptor per (partition, k)
    adj_v = adj_ap.rearrange("b k (p c) m -> b p k c m", c=2)
    # x: natural node chunks (c p): partition p of chunk c = node 128c+p
    x_v = x_ap.rearrange("b (c p) i -> b p c i", p=P)
    # W: partition p holds rows {p, 128+p, 256+p, 384+p} (2KB descriptors)
    w_v = w_ap.rearrange("(ic p) o -> p ic o", p=P)
    # out: partition p holds rows {2p, 2p+1} -> 2KB bf16 per partition
    out_v = out_ap.rearrange("b (p c) o -> b p c o", c=2)

    with tile.TileContext(nc) as tc:
        with (
            tc.tile_pool(name="singles", bufs=1) as singles,
            tc.tile_pool(name="p_adj", bufs=13) as p_adj,
            tc.tile_pool(name="p_x", bufs=4) as p_x,
            tc.tile_pool(name="p_xt", bufs=3) as p_xt,
            tc.tile_pool(name="p_h", bufs=6) as p_h,
            tc.tile_pool(name="p_at", bufs=10) as p_at,
            tc.tile_pool(name="p_out", bufs=4) as p_out,
            tc.tile_pool(name="ps_t", bufs=3, space=bass.MemorySpace.PSUM) as ps_t,
            tc.tile_pool(name="ps_h", bufs=2, space=bass.MemorySpace.PSUM) as ps_h,
            tc.tile_pool(name="ps_o", bufs=2, space=bass.MemorySpace.PSUM) as ps_o,
        ):
            # --- GpSimd SWDGE FIFO: W, bias, then x/adj per batch ---
            adj_sbs = [[None, None] for _ in range(BPC)]
            x_sbs = []
            id_c = singles.tile([P, P], cdt)
            w_all = singles.tile([P, IC4 * OUT_DIM], cdt)
            nc.gpsimd.dma_start(out=w_all[:], in_=w_v)
            bias_c = singles.tile([1, OUT_DIM], cdt)
            nc.gpsimd.dma_start(out=bias_c[:], in_=b_ap[None, :])

            def w_sb(ic):
                return w_all[:, ic * OUT_DIM:(ic + 1) * OUT_DIM]

            def load_adj(b, hf):
                at = p_adj.tile([P, KH * 2 * N], cdt, tag="adj",
                                name=f"a{b}_{hf}")
                nc.gpsimd.dma_start(
                    out=at[:], in_=adj_v[b, :, hf * KH:(hf + 1) * KH])
                adj_sbs[b][hf] = at

            def load_x(b):
                xt = p_x.tile([P, NC2 * IN_DIM], cdt, tag="x", name=f"x{b}")
                nc.gpsimd.dma_start(out=xt[:], in_=x_v[b])
                x_sbs.append(xt)

            load_x(0)
            load_adj(0, 0)
            make_identity(nc, id_c[:])
            load_adj(0, 1)
            for b in range(1, BPC):
                load_x(b)
                load_adj(b, 0)
                load_adj(b, 1)

            # --- DVE: warmup tile + ones row ---
            warm = singles.tile([P, P], cdt)
            nc.vector.memset(warm[:], 0.125)
            ones_row = singles.tile([1, P], cdt)
            nc.vector.memset(ones_row[:], 1.0)

            # --- PE warmup (borrows the ps_o ring) ---
            pw = [ps_o.tile([P, OUT_DIM], FP32, tag="pso", name=f"wm{i}")
                  for i in range(2)]
            for i in range(WARMUP_MM):
                nc.tensor.matmul(pw[i % 2][:, :P], warm[:], warm[:],
                                 start=True, stop=True)

            pend = [None] * BPC
            po_pend = [None] * BPC

            def emit_xt(b):
                """xT transposes for batch b -> xt tile (DVE drain)"""
                x_sb = x_sbs[b]
                pt = ps_t.tile([P, NC2 * IC4 * P], cdt, tag="pst",
                               name=f"ptx{b}")
                for c in range(NC2):
                    for ic in range(IC4):
                        nc.tensor.transpose(
                            pt[:, (c * IC4 + ic) * P:(c * IC4 + ic + 1) * P],
                            x_sb[:, c * IN_DIM + ic * P:
                                 c * IN_DIM + (ic + 1) * P],
                            id_c[:])
                xt_sb = p_xt.tile([P, NC2 * IC4 * P], cdt, tag="xT",
                                  name=f"xT{b}")
                nc.vector.tensor_copy(xt_sb[:], pt[:])
                return xt_sb

            def emit_linear(b, xt_sb):
                h_sb = []
                for c in range(NC2):
                    ph = ps_h.tile([P, OUT_DIM], FP32, tag="psh",
                                   name=f"ph{b}_{c}")
                    nc.tensor.matmul(ph[:], ones_row[:], bias_c[:],
                                     start=True, stop=False)
                    for ic in range(IC4):
                        nc.tensor.matmul(
                            ph[:], xt_sb[:, (c * IC4 + ic) * P:
                                         (c * IC4 + ic + 1) * P],
                            w_sb(ic), start=False, stop=(ic == IC4 - 1))
                    ht = p_h.tile([P, OUT_DIM], cdt, tag="h", name=f"h{b}_{c}")
                    nc.scalar.activation(ht[:], ph[:],
                                         mybir.ActivationFunctionType.Relu)
                    h_sb.append(ht)
                return h_sb

            def emit_adjt(b, half):
                """adjT transposes for the k-pairs of one half"""
                aT = []
                for kph in range(KPH):
                    kp = half * KPH + kph
                    pt = ps_t.tile([P, 8 * P], cdt, tag="pst",
                                   name=f"pta{b}_{kp}")
                    for kk in range(2):
                        k = kp * 2 + kk
                        a_sb = adj_sbs[b][k // KH]
                        klocal = k % KH
                        for c in range(2):
                            for mch in range(NC2):
                                nc.tensor.transpose(
                                    pt[:, (kk * 4 + c * NC2 + mch) * P:
                                       (kk * 4 + c * NC2 + mch + 1) * P],
                                    a_sb[:, klocal * 2 * N + c * N + mch * P:
                                         klocal * 2 * N + c * N + (mch + 1) * P],
                                    id_c[:])
                    t = p_at.tile([P, 8 * P], cdt, tag="aT",
                                  name=f"aT{b}_{kp}")
                    nc.vector.tensor_copy(t[:], pt[:])
                    aT.append(t)
                return aT

            def emit_agg_half(b, half):
                h_sb, aT = pend[b]
                if po_pend[b] is None:
                    po_pend[b] = [ps_o.tile([P, OUT_DIM], FP32, tag="pso",
                                            name=f"po{b}_{c}")
                                  for c in range(2)]
                po = po_pend[b]
                for k in range(half * KH, (half + 1) * KH):
                    kp, kk = k // 2, k % 2
                    for c in range(2):
                        for mch in range(NC2):
                            nc.tensor.matmul(
                                po[c][:, k * DK:(k + 1) * DK],
                                aT[kp][:, (kk * 4 + c * NC2 + mch) * P:
                                       (kk * 4 + c * NC2 + mch + 1) * P],
                                h_sb[mch][:, k * DK:(k + 1) * DK],
                                start=(mch == 0), stop=(mch == NC2 - 1))

            def emit_store(b):
                """parallel po drain (ACT + DVE) then one bf16 store"""
                po = po_pend[b]
                ot = p_out.tile([P, 2 * OUT_DIM], cdt, tag="o", name=f"o{b}")
                nc.scalar.copy(ot[:, :OUT_DIM], po[0][:])
                nc.vector.tensor_copy(ot[:, OUT_DIM:], po[1][:])
                nc.sync.dma_start(out=out_v[b], in_=ot[:])

            # --- slot 0: warmup bridge + front of batch 0 ---
            xt0 = emit_xt(0)
            for i in range(BRIDGE_MM):
                nc.tensor.matmul(pw[i % 2][:], warm[:], w_sb(0),
                                 start=True, stop=True)
            h0 = emit_linear(0, xt0)
            pend[0] = (h0, emit_adjt(0, 0))
            pend[0] = (h0, pend[0][1] + emit_adjt(0, 1))

            # --- steady-state slots ---
            for b in range(1, BPC):
                xt_sb = emit_xt(b)
                emit_agg_half(b - 1, 0)
                h_sb = emit_linear(b, xt_sb)
                emit_agg_half(b - 1, 1)
                emit_store(b - 1)
                pend[b] = (h_sb, emit_adjt(b, 0))
                pend[b] = (h_sb, pend[b][1] + emit_adjt(b, 1))

            # --- tail: batch 7 aggregation ---
            emit_agg_half(BPC - 1, 0)
            emit_agg_half(BPC - 1, 1)
            emit_store(BPC - 1)

    nc.compile()
    return nc


def _get_nc():
    if "nc" not in _compiled:
        _compiled["nc"] = _build()
    return _compiled["nc"]


def _run(inputs, trace=False, trace_cores=None):
    nc = _get_nc()
    node_feats = np.ascontiguousarray(inputs["node_feats"], dtype=np.float32)
    adj = np.ascontiguousarray(inputs["adj"], dtype=np.float32)
    weight = np.ascontiguousarray(inputs["weight"], dtype=np.float32)
    bias = np.ascontiguousarray(inputs["bias"], dtype=np.float32)
    in_maps = []
    for c in range(N_CORES):
        sl = slice(c * BPC, (c + 1) * BPC)
        in_maps.append({
            "node_feats": node_feats[sl],
            "adj": adj[sl],
            "weight": weight,
            "bias": bias,
        })
    res = run_bass_kernel_spmd(
        nc, in_maps, core_ids=list(range(N_CORES)),
        trace=trace, trace_cores=trace_cores)
    out = np.concatenate(
        [np.asarray(res.results[c]["out"]).astype(np.float32)
         for c in range(N_CORES)], axis=0)
    return out.reshape(B, N, OUT_DIM), res


def kernel(**inputs) -> np.ndarray:
    return _run(inputs, trace=False)[0]
